# revision 10
# baseline (speedup 1.0000x reference)
"""Trainium2 Bass kernel for decoder multi-head self-attention (16 heads,
d_model=2048, bs=32, q_len=16, kv_len=2048), tensor-parallel over heads
across 8 NeuronCores (2 heads per core).

Per-core dataflow (all matmuls in float32r):
  - Q/K/V projections from replicated x^T; q kept transposed+banded for
    scores, k_new kept transposed for cache-splice, v_new kept natural.
  - scores[bq, kv] via qT (stationary, 32-col band: 16 q + 16 zero pad)
    x kT (host-pre-transposed cache slice, moving 512-wide).
  - softmax without max-subtraction (values are O(1)); exp+rowsum fused on
    ScalarE via accum_out; normalize on VectorE.
  - attn^T via PE transpose; attn@V with V natural as stationary.
  - output projection on-device against Wo[:, S].T; partials summed on host.
"""

import os
import threading
from concurrent.futures import ThreadPoolExecutor

import numpy as np

N_CORES = 8
N_HEAD = 16
D_K = 128
D_MODEL = 2048
BS = 32
QL = 16
KV = 2048
BQ = BS * QL  # 512
DSH = D_MODEL // N_CORES  # 256 (2 heads per core)
SCALE = 1.0 / np.sqrt(D_K)

_lock = threading.Lock()
_cache = {}


def _patched_tile_context():
    import bass_rust
    import concourse.tile as tile
    from concourse.vector_clock import ScopedClock

    class PatchedTileContext(tile.TileContext):
        # The pinned walrus rejects instructions carrying >2 semaphore waits
        # ("Too many sync wait commands" on the kernel-tail Drain).  Split the
        # drain's waits onto dedicated SP nops, one wait each.
        def _drain_and_barrier(self, tick_clock, wait_clock):
            probe = self.nc.sync.nop(nofuse=True)
            wait_clock.add_sem_waits(
                probe.ins, ScopedClock({None: tick_clock.global_clock})
            )
            si = probe.ins.sync_info
            waits = list(si.on_wait) if si is not None else []
            if len(waits) > 1:
                probe.ins.sync_info = bass_rust.SyncInfo(
                    on_wait=waits[:1], on_update=list(si.on_update)
                )
                for w in waits[1:]:
                    n = self.nc.sync.nop(nofuse=True)
                    n.ins.sync_info = bass_rust.SyncInfo(on_wait=[w], on_update=[])
            self.nc.sync.drain()
            self.nc.all_engine_barrier()
            assert self.sems is not None
            popped = self.nc._tile_sem_poison_stack.pop()
            assert popped is self._sem_poison
            self.nc.clear_and_free_semaphores(list(self.sems.allocated().values()))
            self.nc.all_engine_barrier()

    return PatchedTileContext


def _build(masked: bool, use_f32r: bool = False):
    import concourse.bass as bass
    import concourse.mybir as mybir
    from concourse.masks import make_identity

    TileContext = _patched_tile_context()
    f32 = mybir.dt.float32
    f32r = mybir.dt.float32r if use_f32r else mybir.dt.float32
    EXP = mybir.ActivationFunctionType.Exp
    ADD = mybir.AluOpType.add
    MULT = mybir.AluOpType.mult

    nc = bass.Bass()

    xT = nc.dram_tensor("xT", [D_MODEL, BQ], f32r, kind="ExternalInput")
    wqT = nc.dram_tensor("wqT", [D_MODEL, DSH], f32r, kind="ExternalInput")
    wkT = nc.dram_tensor("wkT", [D_MODEL, DSH], f32r, kind="ExternalInput")
    wvT = nc.dram_tensor("wvT", [D_MODEL, DSH], f32r, kind="ExternalInput")
    woT = nc.dram_tensor("woT", [DSH, D_MODEL], f32r, kind="ExternalInput")
    bq2 = nc.dram_tensor("bq2", [DSH, 1], f32, kind="ExternalInput")
    bk2 = nc.dram_tensor("bk2", [DSH, 1], f32, kind="ExternalInput")
    bvr = nc.dram_tensor("bvr", [1, DSH], f32r, kind="ExternalInput")
    kTd = nc.dram_tensor("kT", [BS, DSH, KV], f32r, kind="ExternalInput")
    vd = nc.dram_tensor("v", [BS, KV, DSH], f32r, kind="ExternalInput")
    if masked:
        sbias = nc.dram_tensor("sbias", [BS * 32, KV], f32, kind="ExternalInput")

    out_p = nc.dram_tensor("out_partial", [BQ, D_MODEL], f32, kind="ExternalOutput")
    debug = os.environ.get("ATTN_DEBUG", "0") == "1"
    if debug:
        dbg_exp = nc.dram_tensor("dbg_exp", [64, KV], f32, kind="ExternalOutput")
        dbg_outT = nc.dram_tensor("dbg_outT", [128, 1024], f32, kind="ExternalOutput")
    knT_o = nc.dram_tensor("k_newT", [DSH, BQ], f32, kind="ExternalOutput")
    vn_o = nc.dram_tensor("v_new", [128, 4 * DSH], f32r, kind="ExternalOutput")

    with TileContext(nc) as tc:
        with (
            tc.tile_pool(name="const", bufs=1) as cpool,
            tc.tile_pool(name="persist", bufs=1) as ppool,
        ):
            # constants
            idt = cpool.tile([128, 128], f32r)
            make_identity(nc, idt[:])
            ones1 = cpool.tile([1, 128], f32r)
            nc.vector.memset(ones1[:], 1.0)
            bq_sb = cpool.tile([128, 2], f32)
            nc.sync.dma_start(
                out=bq_sb[:], in_=bq2[:].rearrange("(h p) o -> p (h o)", p=128)
            )
            bk_sb = cpool.tile([128, 2], f32)
            nc.sync.dma_start(
                out=bk_sb[:], in_=bk2[:].rearrange("(h p) o -> p (h o)", p=128)
            )
            bv_sb = cpool.tile([1, DSH], f32r)
            nc.sync.dma_start(out=bv_sb[:], in_=bvr[:])

            # persistent products of the projection phase
            qT_pad = ppool.tile([128, 2048], f32r)  # (h, b, 32-band) banded, scaled
            nc.vector.memset(qT_pad[:], 0.0)
            knT = ppool.tile([128, 1024], f32)  # (h, b, q)
            vn = ppool.tile([128, 4, DSH], f32r)  # row p, (bq-chunk, dsh)
            bvb = ppool.tile([128, DSH], f32)  # bv broadcast over partitions

            # ---------------- projection phase ----------------
            with (
                tc.tile_pool(name="xp", bufs=3) as xpool,
                tc.tile_pool(name="wp", bufs=2) as wpool,
                tc.tile_pool(name="pp", bufs=1, space="PSUM") as ppsum,
            ):
                q_ps = [ppsum.tile([128, BQ], f32, tag=f"q{h}", name=f"q_ps{h}") for h in range(2)]
                k_ps = [ppsum.tile([128, BQ], f32, tag=f"k{h}", name=f"k_ps{h}") for h in range(2)]
                v_ps = [ppsum.tile([128, DSH], f32, tag=f"v{c}", name=f"v_ps{c}") for c in range(4)]
                for dc in range(16):
                    xt = xpool.tile([128, BQ], f32r)
                    nc.sync.dma_start(out=xt[:], in_=xT[dc * 128 : (dc + 1) * 128, :])
                    wq_t = wpool.tile([128, DSH], f32r, tag="wq")
                    nc.sync.dma_start(
                        out=wq_t[:], in_=wqT[dc * 128 : (dc + 1) * 128, :]
                    )
                    wk_t = wpool.tile([128, DSH], f32r, tag="wk")
                    nc.sync.dma_start(
                        out=wk_t[:], in_=wkT[dc * 128 : (dc + 1) * 128, :]
                    )
                    wv_t = wpool.tile([128, DSH], f32r, tag="wv")
                    nc.sync.dma_start(
                        out=wv_t[:], in_=wvT[dc * 128 : (dc + 1) * 128, :]
                    )
                    st, sp = dc == 0, dc == 15
                    for h in range(2):
                        hs = slice(h * 128, (h + 1) * 128)
                        nc.tensor.matmul(
                            q_ps[h][:],
                            lhsT=wq_t[:, hs],
                            rhs=xt[:],
                            start=st,
                            stop=sp,
                        )
                        nc.tensor.matmul(
                            k_ps[h][:],
                            lhsT=wk_t[:, hs],
                            rhs=xt[:],
                            start=st,
                            stop=sp,
                        )
                    for c4 in range(4):
                        nc.tensor.matmul(
                            v_ps[c4][:],
                            lhsT=xt[:, c4 * 128 : (c4 + 1) * 128],
                            rhs=wv_t[:],
                            start=st,
                            stop=sp,
                        )
                # evacuate qT into banded layout with bias+scale
                qview = qT_pad[:].rearrange("p (h b w) -> p h b w", h=2, b=32, w=32)
                for h in range(2):
                    nc.vector.tensor_scalar(
                        out=qview[:, h, :, 0:QL],
                        in0=q_ps[h][:].rearrange("p (b q) -> p b q", b=32),
                        scalar1=bq_sb[:, h : h + 1],
                        scalar2=float(SCALE),
                        op0=ADD,
                        op1=MULT,
                    )
                    nc.vector.tensor_scalar_add(
                        out=knT[:, h * 512 : (h + 1) * 512],
                        in0=k_ps[h][:],
                        scalar1=bk_sb[:, h : h + 1],
                    )
                # bv broadcast to 128 partitions via PE, then v evac with bias
                bv_ps = ppsum.tile([128, DSH], f32, tag="q0")
                nc.tensor.matmul(
                    bv_ps[:],
                    lhsT=ones1[:],
                    rhs=bv_sb[:],
                    start=True,
                    stop=True,
                )
                nc.vector.tensor_copy(bvb[:], bv_ps[:])
                for c4 in range(4):
                    nc.vector.tensor_tensor(
                        out=vn[:, c4, :], in0=v_ps[c4][:], in1=bvb[:], op=ADD
                    )
                # ship new k/v shards to host early
                for h in range(2):
                    nc.sync.dma_start(
                        out=knT_o[h * 128 : (h + 1) * 128, :],
                        in_=knT[:, h * 512 : (h + 1) * 512],
                    )
                nc.sync.dma_start(
                    out=vn_o[:], in_=vn[:].rearrange("p c f -> p (c f)")
                )

            # ---------------- attention phase ----------------
            with (
                tc.tile_pool(name="kt", bufs=2) as kpool,
                tc.tile_pool(name="vt", bufs=2) as vpool,
                tc.tile_pool(name="et", bufs=2) as epool,
                tc.tile_pool(name="at", bufs=2) as apool,
                tc.tile_pool(name="dn", bufs=2) as dpool,
                tc.tile_pool(name="wo", bufs=1) as wopool,
                tc.tile_pool(name="sps", bufs=1, space="PSUM") as spsum,
                tc.tile_pool(name="tps", bufs=2, space="PSUM") as tpsum,
                tc.tile_pool(name="ops", bufs=1, space="PSUM") as opsum,
            ):
                wo_t = []
                for h in range(2):
                    w = wopool.tile([128, D_MODEL], f32r, tag=f"wo{h}", name=f"wo_t{h}")
                    nc.sync.dma_start(
                        out=w[:], in_=woT[h * 128 : (h + 1) * 128, :]
                    )
                    wo_t.append(w)

                outT_ps = opsum.tile([128, 1024], f32)  # (h, b, q)

                for g in range(16):
                    b0 = 2 * g
                    kts = []
                    vts = []
                    for bi in range(2):
                        b = b0 + bi
                        kh = []
                        for h in range(2):
                            t = kpool.tile([128, KV], f32r, tag=f"k{bi}h{h}", name=f"kt{bi}{h}")
                            nc.sync.dma_start(
                                out=t[:], in_=kTd[b, h * 128 : (h + 1) * 128, :]
                            )
                            # splice new k columns (cache positions 2032..2047)
                            nc.vector.tensor_copy(
                                t[:, KV - QL :],
                                knT[:, h * 512 + b * QL : h * 512 + (b + 1) * QL],
                            )
                            kh.append(t)
                        kts.append(kh)
                        vt = vpool.tile([128, 16, DSH], f32r, tag=f"v{bi}", name=f"vt{bi}")
                        nc.sync.dma_start(
                            out=vt[:], in_=vd[b].rearrange("(c p) f -> p c f", p=128)
                        )
                        # splice new v rows (cache positions 2032..2047)
                        nc.sync.dma_start(
                            out=vt[112:128, 15, :],
                            in_=vn[
                                (b % 8) * 16 : (b % 8) * 16 + 16, b // 8, :
                            ],
                        )
                        vts.append(vt)

                    for h in range(2):
                        sc_ps = spsum.tile([64, KV], f32)
                        for bi in range(2):
                            b = b0 + bi
                            lhs = qT_pad[:, h * 1024 + b * 32 : h * 1024 + (b + 1) * 32]
                            for kc4 in range(4):
                                ks = slice(kc4 * 512, (kc4 + 1) * 512)
                                nc.tensor.matmul(
                                    sc_ps[bi * 32 : (bi + 1) * 32, ks],
                                    lhsT=lhs,
                                    rhs=kts[bi][h][:, ks],
                                    start=True,
                                    stop=True,
                                )
                        if masked:
                            sb_t = epool.tile([64, KV], f32, tag="sb", name="sb_t")
                            nc.sync.dma_start(
                                out=sb_t[:],
                                in_=sbias[b0 * 32 : b0 * 32 + 64, :],
                            )
                            nc.vector.tensor_tensor(
                                out=sc_ps[:], in0=sc_ps[:], in1=sb_t[:], op=ADD
                            )
                        exp_t = epool.tile([64, KV], f32r, tag="exp")
                        den = dpool.tile([64, 1], f32, tag="den")
                        nc.scalar.activation(
                            out=exp_t[:], in_=sc_ps[:], func=EXP, accum_out=den[:]
                        )
                        rec = dpool.tile([64, 1], f32, tag="rec")
                        nc.vector.reciprocal(rec[:], den[:])
                        nc.vector.tensor_scalar_mul(exp_t[:], exp_t[:], rec[:])
                        if debug and g == 0 and h == 0:
                            nc.sync.dma_start(out=dbg_exp[:], in_=exp_t[:])
                        at_list = []
                        for kc in range(16):
                            tp = tpsum.tile([128, 64], f32r)
                            nc.tensor.transpose(
                                tp[:],
                                exp_t[:, kc * 128 : (kc + 1) * 128],
                                idt[0:64, 0:64],
                            )
                            at_t = apool.tile([128, 64], f32r, tag=f"at{kc}")
                            nc.vector.tensor_copy(at_t[:], tp[:])
                            at_list.append(at_t)
                        # each (b, h) accumulation group's matmuls must be
                        # consecutive: a start=True from another group in the
                        # same PSUM bank resets in-flight accumulation state
                        for bi in range(2):
                            b = b0 + bi
                            for kc in range(16):
                                nc.tensor.matmul(
                                    outT_ps[:, h * 512 + b * QL : h * 512 + (b + 1) * QL],
                                    lhsT=vts[bi][:, kc, h * 128 : (h + 1) * 128],
                                    rhs=at_list[kc][:, bi * 32 : bi * 32 + QL],
                                    start=(kc == 0),
                                    stop=(kc == 15),
                                    skip_group_check=True,
                                )

                outT_sb = ppool.tile([128, 1024], f32r)
                nc.vector.tensor_copy(outT_sb[:], outT_ps[:])
                if debug:
                    nc.sync.dma_start(out=dbg_outT[:], in_=outT_sb[:])

            # ---------------- output projection ----------------
            with (
                tc.tile_pool(name="os", bufs=2) as ospool,
                tc.tile_pool(name="wo2", bufs=1) as wopool2,
                tc.tile_pool(name="ops2", bufs=2, space="PSUM") as opsum2,
            ):
                wo2 = []
                for h in range(2):
                    w = wopool2.tile([128, D_MODEL], f32r, tag=f"wo2{h}", name=f"wo2_t{h}")
                    nc.sync.dma_start(out=w[:], in_=woT[h * 128 : (h + 1) * 128, :])
                    wo2.append(w)
                for bqc in range(4):
                    op_ps = opsum2.tile([128, D_MODEL], f32)
                    for h in range(2):
                        lhs = outT_sb[
                            :, h * 512 + bqc * 128 : h * 512 + (bqc + 1) * 128
                        ]
                        for fs in range(4):
                            fsl = slice(fs * 512, (fs + 1) * 512)
                            nc.tensor.matmul(
                                op_ps[:, fsl],
                                lhsT=lhs,
                                rhs=wo2[h][:, fsl],
                                start=(h == 0),
                                stop=(h == 1),
                            )
                    ot = ospool.tile([128, D_MODEL], f32)
                    nc.vector.tensor_copy(ot[:], op_ps[:])
                    nc.sync.dma_start(
                        out=out_p[bqc * 128 : (bqc + 1) * 128, :], in_=ot[:]
                    )

    _split_excess_waits(nc)
    return nc


def _split_excess_waits(nc, max_waits: int = 1):
    # The pinned walrus can encode at most 1 semaphore wait per engine
    # instruction.  Move any excess
    # waits onto same-engine nops inserted immediately before the instruction.
    import bass_rust

    n_split = 0
    for f in nc.m.functions:
        for blk in f.blocks:
            il = blk.instructions
            i = 0
            while i < len(il):
                ins = il[i]
                si = ins.sync_info
                if si is not None:
                    waits = list(si.on_wait)
                    if len(waits) > max_waits:
                        ins.sync_info = bass_rust.SyncInfo(
                            on_wait=waits[: max_waits], on_update=list(si.on_update)
                        )
                        for j, w in enumerate(waits[max_waits:]):
                            nop = bass_rust.InstNoOp(
                                name=f"{ins.name}-wsplit{j}", ins=[], outs=[]
                            )
                            nop.engine = ins.engine
                            nop.bass_nofuse = True
                            nop.sync_info = bass_rust.SyncInfo(
                                on_wait=[w], on_update=[]
                            )
                            il.insert(i, nop)
                            i += 1
                            n_split += 1
                i += 1
    return n_split


def _get_nc(masked: bool):
    use_f32r = os.environ.get("ATTN_F32R", "0") == "1"
    key = ("nc", masked, use_f32r)
    with _lock:
        if key not in _cache:
            _cache[key] = _build(masked, use_f32r)
        return _cache[key]


def _prep_core(c, x2, k_cache, v_cache, Wq, bq, Wk, bk, Wv, bv, Wo, xT_shared):
    S = slice(c * DSH, (c + 1) * DSH)
    m = {
        "xT": xT_shared,
        "wqT": np.ascontiguousarray(Wq[S, :].T),
        "wkT": np.ascontiguousarray(Wk[S, :].T),
        "wvT": np.ascontiguousarray(Wv[S, :].T),
        "woT": np.ascontiguousarray(Wo[:, S].T),
        "bq2": np.ascontiguousarray(bq[S]).reshape(DSH, 1),
        "bk2": np.ascontiguousarray(bk[S]).reshape(DSH, 1),
        "bvr": np.ascontiguousarray(bv[S]).reshape(1, DSH),
        "kT": np.ascontiguousarray(k_cache[:, :, S].transpose(0, 2, 1)),
        "v": np.ascontiguousarray(v_cache[:, :, S]),
    }
    return m


def kernel(x, k_cache, v_cache, mask, Wq, bq, Wk, bk, Wv, bv, Wo, bo):
    from concourse.bass_utils import run_bass_kernel_spmd

    x = np.asarray(x, dtype=np.float32)
    k_cache = np.asarray(k_cache, dtype=np.float32)
    v_cache = np.asarray(v_cache, dtype=np.float32)
    mask = np.asarray(mask)
    Wq, bq = np.asarray(Wq, np.float32), np.asarray(bq, np.float32)
    Wk, bk = np.asarray(Wk, np.float32), np.asarray(bk, np.float32)
    Wv, bv = np.asarray(Wv, np.float32), np.asarray(bv, np.float32)
    Wo, bo = np.asarray(Wo, np.float32), np.asarray(bo, np.float32)

    masked = not bool((mask != 0).all())
    nc = _get_nc(masked)

    x2 = x.reshape(BQ, D_MODEL)
    xT_shared = np.ascontiguousarray(x2.T)

    with ThreadPoolExecutor(max_workers=8) as ex:
        futs = [
            ex.submit(
                _prep_core, c, x2, k_cache, v_cache, Wq, bq, Wk, bk, Wv, bv, Wo,
                xT_shared,
            )
            for c in range(N_CORES)
        ]
        in_maps = [f.result() for f in futs]

    if masked:
        # additive bias in the banded row layout (b*32 + band), band>=16 rows
        # are padding and must stay 0
        sb = np.zeros((BS, 32, KV), np.float32)
        sb[:, :QL, :] = np.where(mask == 0, np.float32(-1e9), np.float32(0.0))
        sb = sb.reshape(BS * 32, KV)
        for m in in_maps:
            m["sbias"] = sb

    res = run_bass_kernel_spmd(nc, in_maps, list(range(N_CORES)))

    # assemble full outputs
    out = res.results[0]["out_partial"].copy()
    for c in range(1, N_CORES):
        out += res.results[c]["out_partial"]
    out += bo
    out = out.reshape(BS, QL, D_MODEL)

    k_out = k_cache.copy()
    v_out = v_cache.copy()
    for c in range(N_CORES):
        S0 = c * DSH
        knT = res.results[c]["k_newT"]  # [256, 512] (h-major, dk x bq)
        for h in range(2):
            k_out[:, KV - QL :, S0 + h * 128 : S0 + (h + 1) * 128] = (
                knT[h * 128 : (h + 1) * 128, :].T.reshape(BS, QL, 128)
            )
        vn = res.results[c]["v_new"]  # [128, 4*256]
        v_out[:, KV - QL :, S0 : S0 + DSH] = (
            vn.reshape(128, 4, DSH).transpose(1, 0, 2).reshape(BS, QL, DSH)
        )
    return out, k_out, v_out
